# revision 2
# baseline (speedup 1.0000x reference)
"""Decagon-style 2-type/4-relation GNN message passing on 8 Trainium2 NeuronCores.

Strategy (graph/data parallel):
  - Nodes of both types row-sharded across 8 cores (6250 rows each, padded
    to 6272). Per layer, every core projects its own h-shard through the 4
    relation weight matrices (fp16); projected message tables are
    AllGathered into per-core DRAM tables viewed as paired rows
    [25088, 128] fp16 (node rows 2q, 2q+1 side by side, 256B each).
  - Edges dst-sharded: core c owns edges whose dst is in its shard, grouped
    by 32-node dst chunk and padded to whole 128-edge tiles (tile counts
    shared across cores = SPMD).
  - Gather: dma_gather ucode (one instruction per 1024 edges = 8 tiles)
    fetches each edge's paired row (256B) by int16 index q = row >> 1.
    The wanted half is selected by parity at matmul time: two one-hot
    matrices S_even/S_odd (ew pre-masked by parity on host) multiply the
    even/odd 64-col halves of the message tile into the same PSUM block.
  - PE matmuls psum[32c:32c+32, :] += S_par.T @ msg_half accumulate
    weighted segment sums for 4 chunks per 128-node output group.
  - ReLU drain feeds (a) fp32 outputs for layers 1/2/5, (b) an fp16
    PE-transpose into a feature-major copy used for next layer's
    projections.
"""

import sys

sys.path.insert(0, "/opt/trn_rl_repo")

import numpy as np

N_NODES = 50000
F_IN = 128
H = 64
NET = 4
N_CORES = 8
CHUNK = 32   # dst nodes per one-hot matmul column block (psum col group)
GRP = 128    # dst nodes per psum group (4 chunks)
P = 128
OPT = 8      # tiles per dma_gather op (1024 indices; >1024 crashes ucode)


def _ceil(a, b):
    return -(-a // b)


def _prep(src, dst, ew, ns, ns_pad):
    """Per-relation, per-core edge grouping. Returns per-relation dict with
    shared (SPMD) tile structure and per-core slot grids."""
    nchunk = ns_pad // CHUNK
    ng = ns_pad // GRP
    rels = []
    for k in range(src.shape[0]):
        per_core = []
        tiles_per_chunk = np.ones(nchunk, np.int64)
        for c in range(N_CORES):
            m = (dst[k] >= c * ns) & (dst[k] < (c + 1) * ns)
            dl = (dst[k][m] - c * ns).astype(np.int64)
            s = src[k][m].astype(np.int64)
            w = ew[k][m]
            ch = dl // CHUNK
            order = np.lexsort((s, ch))  # chunk-major, src-sorted within
            dl, s, w, ch = dl[order], s[order], w[order], ch[order]
            cnt = np.bincount(ch, minlength=nchunk)
            tiles_per_chunk = np.maximum(tiles_per_chunk, _ceil(cnt, P))
            per_core.append((dl, s, w, ch, cnt))
        tile_base = np.concatenate(([0], np.cumsum(tiles_per_chunk)))
        T = int(tile_base[-1])
        gtiles = [int(tile_base[4 * (g + 1)] - tile_base[4 * g]) for g in range(ng)]
        gops = [_ceil(t, OPT) for t in gtiles]
        opbase = np.concatenate(([0], np.cumsum(gops)))
        OPS = int(opbase[-1])
        dlv = np.zeros((N_CORES, P, T), np.float16)
        ewe = np.zeros((N_CORES, P, T), np.float16)
        ewo = np.zeros((N_CORES, P, T), np.float16)
        idxg = np.zeros((N_CORES, P, OPS * 64), np.int16)
        for c in range(N_CORES):
            dl, s, w, ch, cnt = per_core[c]
            idx0 = np.concatenate(([0], np.cumsum(cnt)))
            rank = np.arange(len(ch)) - idx0[ch]
            slots = tile_base[ch] * P + rank
            row = (s // ns) * ns_pad + (s % ns)
            q = np.zeros(T * P, np.int16)
            d = np.zeros(T * P, np.float16)
            we = np.zeros(T * P, np.float16)
            wo = np.zeros(T * P, np.float16)
            q[slots] = (row >> 1).astype(np.int16)
            d[slots] = (dl % CHUNK).astype(np.float16)
            par = (row & 1).astype(np.float16)
            we[slots] = w.astype(np.float16) * (1.0 - par)
            wo[slots] = w.astype(np.float16) * par
            # slot i -> (partition i % 128, tile i // 128)
            dlv[c] = d.reshape(T, P).T
            ewe[c] = we.reshape(T, P).T
            ewo[c] = wo.reshape(T, P).T
            qT = q.reshape(T, P)
            g16 = np.zeros((16, OPS * 64), np.int16)
            for g in range(ng):
                b0 = int(tile_base[4 * g])
                n_t = gtiles[g]
                blk = np.zeros((gops[g] * OPT, P), np.int16)
                blk[:n_t] = qT[b0:b0 + n_t]
                for o in range(gops[g]):
                    flat = blk[o * OPT:(o + 1) * OPT].reshape(OPT * P)
                    col = (int(opbase[g]) + o) * 64
                    g16[:, col:col + 64] = flat.reshape(64, 16).T
            idxg[c] = np.tile(g16, (8, 1))
        rels.append(dict(T=T, tile_base=tile_base, gtiles=gtiles, gops=gops,
                         opbase=opbase, OPS=OPS, dl=dlv, ewe=ewe, ewo=ewo,
                         idx=idxg))
    return rels


def _build(rels, ns_pad, n_layers=5, out_layers=(0, 1, 4)):
    import concourse.bass as bass  # noqa: F401
    import concourse.mybir as mybir
    import concourse.tile as tile
    from concourse import bacc
    from concourse.masks import make_identity

    F16 = mybir.dt.float16
    F32 = mybir.dt.float32
    I16 = mybir.dt.int16
    AF = mybir.ActivationFunctionType
    OP = mybir.AluOpType

    NG = ns_pad // GRP
    NROWS_P = N_CORES * ns_pad // 2   # paired rows in gathered table
    MAXGT = max(max(r["gtiles"]) for r in rels)
    MAXOPS = max(max(r["gops"]) for r in rels)

    nc = bacc.Bacc("TRN2", target_bir_lowering=False, debug=False,
                   num_devices=N_CORES)

    xT = [nc.dram_tensor(f"x{j}T", [F_IN, ns_pad], F16, kind="ExternalInput")
          for j in range(2)]
    w1 = nc.dram_tensor("w1", [NET * F_IN, H], F16, kind="ExternalInput")
    wl = nc.dram_tensor("wl", [(n_layers - 1) * NET * H, H], F16,
                        kind="ExternalInput")
    dl_d = [nc.dram_tensor(f"dl{k}", [P, rels[k]["T"]], F16,
                           kind="ExternalInput") for k in range(NET)]
    ewe_d = [nc.dram_tensor(f"ewe{k}", [P, rels[k]["T"]], F16,
                            kind="ExternalInput") for k in range(NET)]
    ewo_d = [nc.dram_tensor(f"ewo{k}", [P, rels[k]["T"]], F16,
                            kind="ExternalInput") for k in range(NET)]
    idx_d = [nc.dram_tensor(f"idx{k}", [P, rels[k]["OPS"] * 64], I16,
                            kind="ExternalInput") for k in range(NET)]
    outs = [nc.dram_tensor(f"out{t}", [len(out_layers) * P, NG * H], F32,
                           kind="ExternalOutput") for t in range(2)]

    cc_in = [nc.dram_tensor(f"ccin{k}", [ns_pad, H], F16) for k in range(NET)]
    tblp = [nc.dram_tensor(f"tbl{k}", [NROWS_P, 2 * H], F16,
                           addr_space="Shared") for k in range(NET)]

    with tile.TileContext(nc) as tc:
        with (
            tc.tile_pool(name="res", bufs=1) as res,
            tc.tile_pool(name="msg", bufs=3) as msgp,
            tc.tile_pool(name="idxp", bufs=3) as idxp,
            tc.tile_pool(name="sp", bufs=3) as sp,
            tc.tile_pool(name="small", bufs=3) as small,
            tc.tile_pool(name="pagg", bufs=3, space="PSUM") as pagg,
            tc.tile_pool(name="ptr", bufs=2, space="PSUM") as ptr,
            tc.tile_pool(name="ppr", bufs=2, space="PSUM") as ppr,
        ):
            # ---- resident loads ----
            dl_sb = [res.tile([P, rels[k]["T"]], F16, tag=f"dl{k}", name=f"dlsb{k}")
                     for k in range(NET)]
            ewe_sb = [res.tile([P, rels[k]["T"]], F16, tag=f"we{k}", name=f"wesb{k}")
                      for k in range(NET)]
            ewo_sb = [res.tile([P, rels[k]["T"]], F16, tag=f"wo{k}", name=f"wosb{k}")
                      for k in range(NET)]
            for k in range(NET):
                nc.sync.dma_start(dl_sb[k][:], dl_d[k][:])
                nc.sync.dma_start(ewe_sb[k][:], ewe_d[k][:])
                nc.sync.dma_start(ewo_sb[k][:], ewo_d[k][:])
            xT_sb = [res.tile([F_IN, ns_pad], F16, tag=f"xT{j}", name=f"xTsb{j}")
                     for j in range(2)]
            for j in range(2):
                nc.sync.dma_start(xT_sb[j][:], xT[j][:])
            w1_sb = res.tile([F_IN, NET * H], F16, tag="w1", name="w1sb")
            nc.sync.dma_start(
                w1_sb[:].rearrange("p (k f) -> p k f", k=NET),
                w1.ap().rearrange("(k p) f -> p k f", k=NET),
            )
            nwl = (n_layers - 1) * NET
            wl_sb = res.tile([H, nwl * H], F16, tag="wl", name="wlsb")
            nc.sync.dma_start(
                wl_sb[:].rearrange("p (m f) -> p m f", m=nwl),
                wl.ap().rearrange("(m p) f -> p m f", m=nwl),
            )
            ident = res.tile([P, P], F16, tag="ident", name="ident")
            make_identity(nc, ident[:])
            iota_i = res.tile([P, MAXGT * CHUNK], I16, tag="ioi", name="iotai")
            nc.gpsimd.iota(
                iota_i[:].rearrange("p (t c) -> p t c", c=CHUNK),
                pattern=[[0, MAXGT], [1, CHUNK]], base=0, channel_multiplier=0,
            )
            iota_f = res.tile([P, MAXGT * CHUNK], F16, tag="iof", name="iotaf")
            nc.vector.tensor_copy(out=iota_f[:], in_=iota_i[:])
            hT = [res.tile([H, ns_pad], F16, tag=f"hT{j}", name=f"hTsb{j}")
                  for j in range(2)]
            f32st = res.tile([P, NG * H], F32, tag="f32st", name="f32st")

            # ---- layer-1 projections from xT ----
            for g in range(NG):
                pp = ppr.tile([P, NET * H], F32, tag="pp", name="pp",
                              padded_shape=[P, 512])
                for k in range(NET):
                    nc.tensor.matmul(
                        out=pp[:, k * H:(k + 1) * H],
                        lhsT=xT_sb[k % 2][:, g * GRP:(g + 1) * GRP],
                        rhs=w1_sb[:, k * H:(k + 1) * H],
                        start=True, stop=True,
                    )
                tmp = small.tile([P, NET * H], F16, tag="projdr", name="projdr")
                nc.scalar.activation(out=tmp[:], in_=pp[:], func=AF.Copy)
                for k in range(NET):
                    nc.sync.dma_start(
                        cc_in[k].ap().rearrange("(t p) f -> p t f", p=P)[:, g, :],
                        tmp[:, k * H:(k + 1) * H],
                    )

            # ---- layers ----
            for L in range(n_layers):
                relu = L < n_layers - 1
                for k in range(NET):
                    nc.gpsimd.collective_compute(
                        "AllGather", OP.bypass,
                        replica_groups=[list(range(N_CORES))],
                        ins=[cc_in[k].ap().opt()], outs=[tblp[k].ap().opt()],
                    )
                for it in range(2):
                    rels_it = (2 * it, 2 * it + 1)
                    for g in range(NG):
                        mbufs, se_b, so_b = {}, {}, {}
                        for k in rels_it:
                            r = rels[k]
                            b0 = int(r["tile_base"][4 * g])
                            n_t = r["gtiles"][g]
                            nops = r["gops"][g]
                            ob = int(r["opbase"][g])
                            idx_t = idxp.tile([P, MAXOPS * 64], I16,
                                              tag=f"ix{k % 2}", name=f"ix{k % 2}")
                            nc.sync.dma_start(
                                idx_t[:, :nops * 64],
                                idx_d[k][:, ob * 64:(ob + nops) * 64])
                            mb = msgp.tile([P, MAXOPS * OPT * 2 * H], F16,
                                           tag=f"m{k % 2}", name=f"mb{k % 2}")
                            for o in range(nops):
                                nc.gpsimd.dma_gather(
                                    out_ap=mb[:, o * OPT * 2 * H:(o + 1) * OPT * 2 * H]
                                    .rearrange("p (t e) -> p t e", e=2 * H),
                                    in_ap=tblp[k][:],
                                    idxs_ap=idx_t[:, o * 64:(o + 1) * 64],
                                    num_idxs=OPT * P,
                                    num_idxs_reg=OPT * P,
                                    elem_size=2 * H,
                                )
                            oh = sp.tile([P, MAXGT * CHUNK], F16,
                                         tag=f"oh{k % 2}", name=f"ohb{k % 2}")
                            s_e = sp.tile([P, MAXGT * CHUNK], F16,
                                          tag=f"Se{k % 2}", name=f"Se{k % 2}")
                            s_o = sp.tile([P, MAXGT * CHUNK], F16,
                                          tag=f"So{k % 2}", name=f"So{k % 2}")
                            dlb = dl_sb[k][:, b0:b0 + n_t].to_broadcast(
                                [P, n_t, CHUNK])
                            i3 = iota_f[:, :n_t * CHUNK].rearrange(
                                "p (t c) -> p t c", c=CHUNK)
                            nc.vector.tensor_tensor(
                                out=oh[:, :n_t * CHUNK].rearrange(
                                    "p (t c) -> p t c", c=CHUNK),
                                in0=i3, in1=dlb, op=OP.is_equal)
                            for s_buf, ew_sb in ((s_e, ewe_sb), (s_o, ewo_sb)):
                                nc.vector.tensor_tensor(
                                    out=s_buf[:, :n_t * CHUNK].rearrange(
                                        "p (t c) -> p t c", c=CHUNK),
                                    in0=oh[:, :n_t * CHUNK].rearrange(
                                        "p (t c) -> p t c", c=CHUNK),
                                    in1=ew_sb[k][:, b0:b0 + n_t].to_broadcast(
                                        [P, n_t, CHUNK]),
                                    op=OP.mult)
                            mbufs[k], se_b[k], so_b[k] = mb, s_e, s_o
                        # matmul schedule: ch4-major so only one psum
                        # accumulation group is open at a time
                        sched = []
                        for ch4 in range(4):
                            for k in rels_it:
                                tb = rels[k]["tile_base"]
                                b0 = int(tb[4 * g])
                                for t in range(int(tb[4 * g + ch4]) - b0,
                                               int(tb[4 * g + ch4 + 1]) - b0):
                                    sched.append((k, t, ch4))
                        first = [True] * 4
                        last_idx = {}
                        for i, (_, _, ch4) in enumerate(sched):
                            last_idx[ch4] = i
                        pt = pagg.tile([P, H], F32, tag="agg", name="pagt",
                                       padded_shape=[P, 512])
                        for i, (k, t, ch4) in enumerate(sched):
                            nc.tensor.matmul(
                                out=pt[ch4 * CHUNK:(ch4 + 1) * CHUNK, :],
                                lhsT=se_b[k][:, t * CHUNK:(t + 1) * CHUNK],
                                rhs=mbufs[k][:, t * 2 * H:t * 2 * H + H],
                                start=first[ch4], stop=False,
                                tile_position=(0, ch4 * CHUNK),
                            )
                            nc.tensor.matmul(
                                out=pt[ch4 * CHUNK:(ch4 + 1) * CHUNK, :],
                                lhsT=so_b[k][:, t * CHUNK:(t + 1) * CHUNK],
                                rhs=mbufs[k][:, t * 2 * H + H:(t + 1) * 2 * H],
                                start=False, stop=(last_idx[ch4] == i),
                                tile_position=(0, ch4 * CHUNK),
                            )
                            first[ch4] = False
                        # drains
                        if L in out_layers:
                            if relu:
                                nc.vector.tensor_scalar_max(
                                    f32st[:, g * H:(g + 1) * H], pt[:], 0.0)
                            else:
                                nc.vector.tensor_copy(
                                    out=f32st[:, g * H:(g + 1) * H], in_=pt[:])
                        if L < n_layers - 1:
                            hr = small.tile([P, H], F16, tag="hr", name="hr")
                            nc.scalar.activation(out=hr[:], in_=pt[:],
                                                 func=AF.Relu)
                            pt2 = ptr.tile([H, P], F16, tag="tr", name="ptt",
                                           padded_shape=[H, 1024])
                            nc.tensor.matmul(out=pt2[:], lhsT=hr[:],
                                             rhs=ident[:], is_transpose=True,
                                             start=True, stop=True)
                            nc.vector.tensor_copy(
                                out=hT[it][:, g * GRP:(g + 1) * GRP],
                                in_=pt2[:])
                    if L in out_layers:
                        sec = out_layers.index(L)
                        nc.sync.dma_start(
                            outs[it][sec * P:(sec + 1) * P, :], f32st[:])
                # next-layer projections
                if L < n_layers - 1:
                    for g in range(NG):
                        pp = ppr.tile([P, NET * H], F32, tag="pp", name="pp",
                                      padded_shape=[P, 512])
                        for k in range(NET):
                            m = L * NET + k
                            nc.tensor.matmul(
                                out=pp[:, k * H:(k + 1) * H],
                                lhsT=hT[k % 2][:, g * GRP:(g + 1) * GRP],
                                rhs=wl_sb[:, m * H:(m + 1) * H],
                                start=True, stop=True,
                            )
                        tmp = small.tile([P, NET * H], F16, tag="projdr",
                                         name="projdr")
                        nc.scalar.activation(out=tmp[:], in_=pp[:], func=AF.Copy)
                        for k in range(NET):
                            nc.sync.dma_start(
                                cc_in[k].ap().rearrange(
                                    "(t p) f -> p t f", p=P)[:, g, :],
                                tmp[:, k * H:(k + 1) * H],
                            )
    nc.compile()
    return nc


def _host_inputs(x0, x1, W1, Wl, rels, ns, ns_pad, n_layers=5):
    xs = [np.asarray(x0), np.asarray(x1)]
    in_maps = []
    for c in range(N_CORES):
        m = {}
        for j in range(2):
            sh = np.zeros((F_IN, ns_pad), np.float16)
            sh[:, :ns] = xs[j][c * ns:(c + 1) * ns].T.astype(np.float16)
            m[f"x{j}T"] = sh
        m["w1"] = np.asarray(W1).reshape(NET * F_IN, H).astype(np.float16)
        m["wl"] = (np.asarray(Wl)[: n_layers - 1]
                   .reshape((n_layers - 1) * NET * H, H).astype(np.float16))
        for k in range(NET):
            m[f"dl{k}"] = rels[k]["dl"][c]
            m[f"ewe{k}"] = rels[k]["ewe"][c]
            m[f"ewo{k}"] = rels[k]["ewo"][c]
            m[f"idx{k}"] = rels[k]["idx"][c]
        in_maps.append(m)
    return in_maps


def _assemble(results, ns, ns_pad, n_out=3):
    NG = ns_pad // GRP
    out = np.zeros((2, N_CORES * ns, n_out * H), np.float32)
    for t in range(2):
        for c in range(N_CORES):
            arr = results[c][f"out{t}"]
            for s in range(n_out):
                a = (arr[s * P:(s + 1) * P]
                     .reshape(P, NG, H).transpose(1, 0, 2).reshape(NG * P, H))
                out[t, c * ns:(c + 1) * ns, s * H:(s + 1) * H] = a[:ns]
    return out


def kernel(x0, x1, src, dst, ew, W1, Wl):
    from concourse.bass_utils import run_bass_kernel_spmd

    x0 = np.asarray(x0); x1 = np.asarray(x1)
    src = np.asarray(src); dst = np.asarray(dst); ew = np.asarray(ew)
    W1 = np.asarray(W1); Wl = np.asarray(Wl)

    ns = x0.shape[0] // N_CORES
    ns_pad = _ceil(ns, GRP) * GRP
    rels = _prep(src, dst, ew, ns, ns_pad)
    nc = _build(rels, ns_pad)
    in_maps = _host_inputs(x0, x1, W1, Wl, rels, ns, ns_pad)
    global _last
    _last = (nc, in_maps, ns, ns_pad)
    res = run_bass_kernel_spmd(nc, in_maps, core_ids=list(range(N_CORES)))
    return _assemble(res.results, ns, ns_pad)


# revision 19
# speedup vs baseline: 2.9601x; 2.9601x over previous
"""Decagon-style 2-type/4-relation GNN message passing on 8 Trainium2 NeuronCores.

Strategy (graph/data parallel):
  - Nodes of both types row-sharded across 8 cores (6250 rows each, padded
    to 6272). Per layer, every core projects its own h-shard through the 4
    relation weight matrices (fp16); projected message tables are
    AllGathered into per-core DRAM tables viewed as paired rows
    [25088, 128] fp16 (node rows 2q, 2q+1 side by side, 256B each).
  - Edges dst-sharded: core c owns edges whose dst is in its shard, grouped
    by 32-node dst chunk and padded to whole 128-edge tiles (tile counts
    shared across cores = SPMD).
  - Gather: dma_gather ucode (one instruction per 1024 edges = 8 tiles)
    fetches each edge's paired row (256B) by int16 index q = row >> 1.
    The wanted half is selected by parity at matmul time: two one-hot
    matrices S_even/S_odd (ew pre-masked by parity on host) multiply the
    even/odd 64-col halves of the message tile into the same PSUM block.
  - PE matmuls psum[32c:32c+32, :] += S_par.T @ msg_half accumulate
    weighted segment sums for 4 chunks per 128-node output group.
  - ReLU drain feeds (a) fp32 outputs for layers 1/2/5, (b) an fp16
    PE-transpose into a feature-major copy used for next layer's
    projections.
"""

import sys

sys.path.insert(0, "/opt/trn_rl_repo")

import numpy as np

N_NODES = 50000
F_IN = 128
H = 64
NET = 4
N_CORES = 8
CHUNK = 32   # dst nodes per one-hot matmul column block (psum col group)
GRP = 128    # dst nodes per psum group (4 chunks)
P = 128
OPT = 8      # tiles per dma_gather op (1024 indices; >1024 crashes ucode)


def _ceil(a, b):
    return -(-a // b)


def _prep(src, dst, ew, ns, ns_pad):
    """Per-relation, per-core edge grouping. Returns per-relation dict with
    shared (SPMD) tile structure and per-core slot grids."""
    nchunk = ns_pad // CHUNK
    ng = ns_pad // GRP
    rels = []
    for k in range(src.shape[0]):
        per_core = []
        tiles_per_chunk = np.ones(nchunk, np.int64)
        for c in range(N_CORES):
            m = (dst[k] >= c * ns) & (dst[k] < (c + 1) * ns)
            dl = (dst[k][m] - c * ns).astype(np.int64)
            s = src[k][m].astype(np.int64)
            w = ew[k][m]
            ch = dl // CHUNK
            order = np.lexsort((s, ch))  # chunk-major, src-sorted within
            dl, s, w, ch = dl[order], s[order], w[order], ch[order]
            cnt = np.bincount(ch, minlength=nchunk)
            tiles_per_chunk = np.maximum(tiles_per_chunk, _ceil(cnt, P))
            per_core.append((dl, s, w, ch, cnt))
        tile_base = np.concatenate(([0], np.cumsum(tiles_per_chunk)))
        T = int(tile_base[-1])
        gtiles = [int(tile_base[4 * (g + 1)] - tile_base[4 * g]) for g in range(ng)]
        gops = [_ceil(t, OPT) for t in gtiles]
        opbase = np.concatenate(([0], np.cumsum(gops)))
        OPS = int(opbase[-1])
        dlv = np.zeros((N_CORES, P, T), np.float16)
        ewe = np.zeros((N_CORES, P, T), np.float16)
        ewo = np.zeros((N_CORES, P, T), np.float16)
        idxg = np.zeros((N_CORES, P, OPS * 64), np.int16)
        for c in range(N_CORES):
            dl, s, w, ch, cnt = per_core[c]
            idx0 = np.concatenate(([0], np.cumsum(cnt)))
            rank = np.arange(len(ch)) - idx0[ch]
            slots = tile_base[ch] * P + rank
            row = (s // ns) * ns_pad + (s % ns)
            q = np.zeros(T * P, np.int16)
            d = np.zeros(T * P, np.float16)
            we = np.zeros(T * P, np.float16)
            wo = np.zeros(T * P, np.float16)
            q[slots] = (row >> 1).astype(np.int16)
            d[slots] = (dl % CHUNK).astype(np.float16)
            par = (row & 1).astype(np.float16)
            we[slots] = w.astype(np.float16) * (1.0 - par)
            wo[slots] = w.astype(np.float16) * par
            # slot i -> (partition i % 128, tile i // 128)
            dlv[c] = d.reshape(T, P).T
            ewe[c] = we.reshape(T, P).T
            ewo[c] = wo.reshape(T, P).T
            qT = q.reshape(T, P)
            g16 = np.zeros((16, OPS * 64), np.int16)
            for g in range(ng):
                b0 = int(tile_base[4 * g])
                n_t = gtiles[g]
                blk = np.zeros((gops[g] * OPT, P), np.int16)
                blk[:n_t] = qT[b0:b0 + n_t]
                for o in range(gops[g]):
                    flat = blk[o * OPT:(o + 1) * OPT].reshape(OPT * P)
                    col = (int(opbase[g]) + o) * 64
                    g16[:, col:col + 64] = flat.reshape(64, 16).T
            idxg[c] = np.tile(g16, (8, 1))
        rels.append(dict(T=T, tile_base=tile_base, gtiles=gtiles, gops=gops,
                         opbase=opbase, OPS=OPS, dl=dlv, ewe=ewe, ewo=ewo,
                         idx=idxg))
    return rels


def _build(rels, ns_pad, n_layers=5, out_layers=(0, 1, 4), skip_gather=False,
           skip_ag=False, skip_mm=False):
    import concourse.bass as bass  # noqa: F401
    import concourse.mybir as mybir
    import concourse.tile as tile
    from concourse import bacc
    from concourse.masks import make_identity

    F16 = mybir.dt.float16
    F32 = mybir.dt.float32
    I16 = mybir.dt.int16
    AF = mybir.ActivationFunctionType
    OP = mybir.AluOpType

    NG = ns_pad // GRP
    NROWS_P = N_CORES * ns_pad // 2   # paired rows in gathered table
    MAXGT = max(max(r["gtiles"]) for r in rels)
    MAXOPS = max(max(r["gops"]) for r in rels)

    nc = bacc.Bacc("TRN2", target_bir_lowering=False, debug=False,
                   num_devices=N_CORES, num_swdge_queues=4)

    xT = [nc.dram_tensor(f"x{j}T", [F_IN, ns_pad], F16, kind="ExternalInput")
          for j in range(2)]
    w1 = nc.dram_tensor("w1", [NET * F_IN, H], F16, kind="ExternalInput")
    wl = nc.dram_tensor("wl", [(n_layers - 1) * NET * H, H], F16,
                        kind="ExternalInput")
    dl_d = [nc.dram_tensor(f"dl{k}", [P, rels[k]["T"]], F16,
                           kind="ExternalInput") for k in range(NET)]
    ewe_d = [nc.dram_tensor(f"ewe{k}", [P, rels[k]["T"]], F16,
                            kind="ExternalInput") for k in range(NET)]
    ewo_d = [nc.dram_tensor(f"ewo{k}", [P, rels[k]["T"]], F16,
                            kind="ExternalInput") for k in range(NET)]
    idx_d = [nc.dram_tensor(f"idx{k}", [P, rels[k]["OPS"] * 64], I16,
                            kind="ExternalInput") for k in range(NET)]
    outs = [nc.dram_tensor(f"out{t}", [len(out_layers) * P, NG * H], F32,
                           kind="ExternalOutput") for t in range(2)]

    cc_in = [nc.dram_tensor(f"ccin{k}", [ns_pad, H], F16) for k in range(NET)]
    tblp = [nc.dram_tensor(f"tbl{k}", [NROWS_P, 2 * H], F16,
                           addr_space="Shared") for k in range(NET)]
    # gathers from Shared DRAM are ~60% slower; copy AG output to local DRAM
    tbll = [nc.dram_tensor(f"tbll{k}", [NROWS_P, 2 * H], F16)
            for k in range(NET)]

    with tile.TileContext(nc) as tc:
        with (
            tc.tile_pool(name="res", bufs=1) as res,
            tc.tile_pool(name="msg", bufs=4) as msgp,
            tc.tile_pool(name="idxp", bufs=4) as idxp,
            tc.tile_pool(name="sp", bufs=3) as sp,
            tc.tile_pool(name="small", bufs=3) as small,
            tc.tile_pool(name="pagg", bufs=3, space="PSUM") as pagg,
            tc.tile_pool(name="ptr", bufs=2, space="PSUM") as ptr,
            tc.tile_pool(name="ppr", bufs=2, space="PSUM") as ppr,
        ):
            # ---- resident loads ----
            dl_sb = [res.tile([P, rels[k]["T"]], F16, tag=f"dl{k}", name=f"dlsb{k}")
                     for k in range(NET)]
            ewe_sb = [res.tile([P, rels[k]["T"]], F16, tag=f"we{k}", name=f"wesb{k}")
                      for k in range(NET)]
            ewo_sb = [res.tile([P, rels[k]["T"]], F16, tag=f"wo{k}", name=f"wosb{k}")
                      for k in range(NET)]
            for k in range(NET):
                nc.sync.dma_start(dl_sb[k][:], dl_d[k][:])
                nc.sync.dma_start(ewe_sb[k][:], ewe_d[k][:])
                nc.sync.dma_start(ewo_sb[k][:], ewo_d[k][:])
            xT_sb = [res.tile([F_IN, ns_pad], F16, tag=f"xT{j}", name=f"xTsb{j}")
                     for j in range(2)]
            for j in range(2):
                nc.sync.dma_start(xT_sb[j][:], xT[j][:])
            w1_sb = res.tile([F_IN, NET * H], F16, tag="w1", name="w1sb")
            nc.sync.dma_start(
                w1_sb[:].rearrange("p (k f) -> p k f", k=NET),
                w1.ap().rearrange("(k p) f -> p k f", k=NET),
            )
            nwl = (n_layers - 1) * NET
            wl_sb = res.tile([H, nwl * H], F16, tag="wl", name="wlsb")
            nc.sync.dma_start(
                wl_sb[:].rearrange("p (m f) -> p m f", m=nwl),
                wl.ap().rearrange("(m p) f -> p m f", m=nwl),
            )
            ident = res.tile([P, P], F16, tag="ident", name="ident")
            make_identity(nc, ident[:])
            iota_i = res.tile([P, MAXGT * CHUNK], I16, tag="ioi", name="iotai")
            nc.gpsimd.iota(
                iota_i[:].rearrange("p (t c) -> p t c", c=CHUNK),
                pattern=[[0, MAXGT], [1, CHUNK]], base=0, channel_multiplier=0,
            )
            iota_f = res.tile([P, MAXGT * CHUNK], F16, tag="iof", name="iotaf")
            nc.vector.tensor_copy(out=iota_f[:], in_=iota_i[:])
            hT = [res.tile([H, ns_pad], F16, tag=f"hT{j}", name=f"hTsb{j}")
                  for j in range(2)]
            f32st = res.tile([P, NG * H], F32, tag="f32st", name="f32st")

            # ---- layer-1 projections from xT ----
            for g in range(NG):
                pp = ppr.tile([P, NET * H], F32, tag="pp", name="pp",
                              padded_shape=[P, 512])
                for k in range(NET):
                    nc.tensor.matmul(
                        out=pp[:, k * H:(k + 1) * H],
                        lhsT=xT_sb[k % 2][:, g * GRP:(g + 1) * GRP],
                        rhs=w1_sb[:, k * H:(k + 1) * H],
                        start=True, stop=True,
                    )
                tmp = small.tile([P, NET * H], F16, tag="projdr", name="projdr")
                nc.scalar.activation(out=tmp[:], in_=pp[:], func=AF.Copy)
                for k in range(NET):
                    nc.sync.dma_start(
                        cc_in[k].ap().rearrange("(t p) f -> p t f", p=P)[:, g, :],
                        tmp[:, k * H:(k + 1) * H],
                    )

            # ---- layers ----
            for L in range(n_layers):
                relu = L < n_layers - 1
                for k in range(NET if not skip_ag else 0):
                    nc.gpsimd.collective_compute(
                        "AllGather", OP.bypass,
                        replica_groups=[list(range(N_CORES))],
                        ins=[cc_in[k].ap().opt()], outs=[tblp[k].ap().opt()],
                    )
                for k in range(NET):
                    nc.sync.dma_start(tbll[k].ap(), tblp[k].ap())
                for it in range(2):
                    rels_it = (2 * it, 2 * it + 1)
                    qctr = 0
                    for g in range(NG):
                        mbufs, se_b, so_b = {}, {}, {}
                        for k in rels_it:
                            r = rels[k]
                            b0 = int(r["tile_base"][4 * g])
                            n_t = r["gtiles"][g]
                            nops = r["gops"][g]
                            ob = int(r["opbase"][g])
                            idx_t = idxp.tile([P, MAXOPS * 64], I16,
                                              tag=f"ix{k % 2}", name=f"ix{k % 2}")
                            nc.sync.dma_start(
                                idx_t[:, :nops * 64],
                                idx_d[k][:, ob * 64:(ob + nops) * 64])
                            mb = msgp.tile([P, MAXOPS * OPT * 2 * H], F16,
                                           tag=f"m{k % 2}", name=f"mb{k % 2}")
                            for o in range(nops):
                                rt = min(OPT, n_t - o * OPT)
                                if skip_gather:
                                    # comparator: same bytes/descriptor shape,
                                    # sequential addresses via HWDGE
                                    nc.sync.dma_start(
                                        mb[:, o * OPT * 2 * H:(o + 1) * OPT * 2 * H]
                                        .rearrange("p (c e) -> p c e", e=2 * H),
                                        tbll[k].ap()[o * 1024:(o + 1) * 1024, :]
                                        .rearrange("(c p) e -> p c e", p=P),
                                    )
                                    continue
                                nc.gpsimd.dma_gather(
                                    out_ap=mb[:, o * OPT * 2 * H:o * OPT * 2 * H + rt * 2 * H]
                                    .rearrange("p (t e) -> p t e", e=2 * H),
                                    in_ap=tbll[k][:],
                                    idxs_ap=idx_t[:, o * 64:o * 64 + rt * 8],
                                    num_idxs=rt * P,
                                    num_idxs_reg=rt * P,
                                    elem_size=2 * H,
                                    queue_num=qctr % 4,
                                )
                                qctr += 1
                            oh = sp.tile([P, MAXGT * CHUNK], F16,
                                         tag=f"oh{k % 2}", name=f"ohb{k % 2}")
                            s_e = sp.tile([P, MAXGT * CHUNK], F16,
                                          tag=f"Se{k % 2}", name=f"Se{k % 2}")
                            s_o = sp.tile([P, MAXGT * CHUNK], F16,
                                          tag=f"So{k % 2}", name=f"So{k % 2}")
                            dlb = dl_sb[k][:, b0:b0 + n_t].to_broadcast(
                                [P, n_t, CHUNK])
                            i3 = iota_f[:, :n_t * CHUNK].rearrange(
                                "p (t c) -> p t c", c=CHUNK)
                            nc.vector.tensor_tensor(
                                out=oh[:, :n_t * CHUNK].rearrange(
                                    "p (t c) -> p t c", c=CHUNK),
                                in0=i3, in1=dlb, op=OP.is_equal)
                            for s_buf, ew_sb in ((s_e, ewe_sb), (s_o, ewo_sb)):
                                nc.vector.tensor_tensor(
                                    out=s_buf[:, :n_t * CHUNK].rearrange(
                                        "p (t c) -> p t c", c=CHUNK),
                                    in0=oh[:, :n_t * CHUNK].rearrange(
                                        "p (t c) -> p t c", c=CHUNK),
                                    in1=ew_sb[k][:, b0:b0 + n_t].to_broadcast(
                                        [P, n_t, CHUNK]),
                                    op=OP.mult)
                            mbufs[k], se_b[k], so_b[k] = mb, s_e, s_o
                        # matmul schedule: ch4-major so only one psum
                        # accumulation group is open at a time
                        sched = []
                        for ch4 in range(4):
                            for k in rels_it:
                                tb = rels[k]["tile_base"]
                                b0 = int(tb[4 * g])
                                for t in range(int(tb[4 * g + ch4]) - b0,
                                               int(tb[4 * g + ch4 + 1]) - b0):
                                    sched.append((k, t, ch4))
                        first = [True] * 4
                        last_idx = {}
                        for i, (_, _, ch4) in enumerate(sched):
                            last_idx[ch4] = i
                        pt = pagg.tile([P, H], F32, tag="agg", name="pagt",
                                       padded_shape=[P, 512])
                        if skip_mm:
                            sched = [s for i, s in enumerate(sched)
                                     if s[2] != sched[i - 1][2] or i == 0]
                            last_idx = {}
                            for i, (_, _, ch4) in enumerate(sched):
                                last_idx[ch4] = i
                        for i, (k, t, ch4) in enumerate(sched):
                            nc.tensor.matmul(
                                out=pt[ch4 * CHUNK:(ch4 + 1) * CHUNK, :],
                                lhsT=se_b[k][:, t * CHUNK:(t + 1) * CHUNK],
                                rhs=mbufs[k][:, t * 2 * H:t * 2 * H + H],
                                start=first[ch4], stop=False,
                                tile_position=(0, ch4 * CHUNK),
                            )
                            nc.tensor.matmul(
                                out=pt[ch4 * CHUNK:(ch4 + 1) * CHUNK, :],
                                lhsT=so_b[k][:, t * CHUNK:(t + 1) * CHUNK],
                                rhs=mbufs[k][:, t * 2 * H + H:(t + 1) * 2 * H],
                                start=False, stop=(last_idx[ch4] == i),
                                tile_position=(0, ch4 * CHUNK),
                            )
                            first[ch4] = False
                        # drains
                        if L in out_layers:
                            if relu:
                                nc.vector.tensor_scalar_max(
                                    f32st[:, g * H:(g + 1) * H], pt[:], 0.0)
                            else:
                                nc.vector.tensor_copy(
                                    out=f32st[:, g * H:(g + 1) * H], in_=pt[:])
                        if L < n_layers - 1:
                            hr = small.tile([P, H], F16, tag="hr", name="hr")
                            nc.scalar.activation(out=hr[:], in_=pt[:],
                                                 func=AF.Relu)
                            pt2 = ptr.tile([H, P], F16, tag="tr", name="ptt",
                                           padded_shape=[H, 1024])
                            nc.tensor.matmul(out=pt2[:], lhsT=hr[:],
                                             rhs=ident[:], is_transpose=True,
                                             start=True, stop=True)
                            nc.vector.tensor_copy(
                                out=hT[it][:, g * GRP:(g + 1) * GRP],
                                in_=pt2[:])
                    if L in out_layers:
                        sec = out_layers.index(L)
                        nc.sync.dma_start(
                            outs[it][sec * P:(sec + 1) * P, :], f32st[:])
                # next-layer projections
                if L < n_layers - 1:
                    for g in range(NG):
                        pp = ppr.tile([P, NET * H], F32, tag="pp", name="pp",
                                      padded_shape=[P, 512])
                        for k in range(NET):
                            m = L * NET + k
                            nc.tensor.matmul(
                                out=pp[:, k * H:(k + 1) * H],
                                lhsT=hT[k % 2][:, g * GRP:(g + 1) * GRP],
                                rhs=wl_sb[:, m * H:(m + 1) * H],
                                start=True, stop=True,
                            )
                        tmp = small.tile([P, NET * H], F16, tag="projdr",
                                         name="projdr")
                        nc.scalar.activation(out=tmp[:], in_=pp[:], func=AF.Copy)
                        for k in range(NET):
                            nc.sync.dma_start(
                                cc_in[k].ap().rearrange(
                                    "(t p) f -> p t f", p=P)[:, g, :],
                                tmp[:, k * H:(k + 1) * H],
                            )
    nc.compile()
    return nc


def _host_inputs(x0, x1, W1, Wl, rels, ns, ns_pad, n_layers=5):
    xs = [np.asarray(x0), np.asarray(x1)]
    in_maps = []
    for c in range(N_CORES):
        m = {}
        for j in range(2):
            sh = np.zeros((F_IN, ns_pad), np.float16)
            sh[:, :ns] = xs[j][c * ns:(c + 1) * ns].T.astype(np.float16)
            m[f"x{j}T"] = sh
        m["w1"] = np.asarray(W1).reshape(NET * F_IN, H).astype(np.float16)
        m["wl"] = (np.asarray(Wl)[: n_layers - 1]
                   .reshape((n_layers - 1) * NET * H, H).astype(np.float16))
        for k in range(NET):
            m[f"dl{k}"] = rels[k]["dl"][c]
            m[f"ewe{k}"] = rels[k]["ewe"][c]
            m[f"ewo{k}"] = rels[k]["ewo"][c]
            m[f"idx{k}"] = rels[k]["idx"][c]
        in_maps.append(m)
    return in_maps


def _assemble(results, ns, ns_pad, n_out=3):
    NG = ns_pad // GRP
    out = np.zeros((2, N_CORES * ns, n_out * H), np.float32)
    for t in range(2):
        for c in range(N_CORES):
            arr = results[c][f"out{t}"]
            for s in range(n_out):
                a = (arr[s * P:(s + 1) * P]
                     .reshape(P, NG, H).transpose(1, 0, 2).reshape(NG * P, H))
                out[t, c * ns:(c + 1) * ns, s * H:(s + 1) * H] = a[:ns]
    return out


def kernel(x0, x1, src, dst, ew, W1, Wl):
    from concourse.bass_utils import run_bass_kernel_spmd

    x0 = np.asarray(x0); x1 = np.asarray(x1)
    src = np.asarray(src); dst = np.asarray(dst); ew = np.asarray(ew)
    W1 = np.asarray(W1); Wl = np.asarray(Wl)

    ns = x0.shape[0] // N_CORES
    ns_pad = _ceil(ns, GRP) * GRP
    rels = _prep(src, dst, ew, ns, ns_pad)
    nc = _build(rels, ns_pad)
    in_maps = _host_inputs(x0, x1, W1, Wl, rels, ns, ns_pad)
    global _last
    _last = (nc, in_maps, ns, ns_pad)
    res = run_bass_kernel_spmd(nc, in_maps, core_ids=list(range(N_CORES)))
    return _assemble(res.results, ns, ns_pad)
